# revision 1
# baseline (speedup 1.0000x reference)
"""Trainium2 Bass kernel: 24x24-bit array multiplier (bit-vector in/out).

Inputs  A, B: [131072, 24] f32 {0,1} bits, LSB-first.
Output: [131072, 48] f32 {0,1} product bits, LSB-first.

Strategy: pure data-parallel across 8 NeuronCores (16384 rows each). Per core,
exact limb arithmetic replaces the reference's bit-level ripple adder:

  1. Horner scan on DVE (tensor_tensor_scan, data0 = {0,.5,...,.5} per 12-bit
     group) turns bits into scaled 12-bit limbs L_l * 2^-11 in one pass.
  2. Limb gathers on ScalarE (activation Copy, scale 2^11, f32->int32).
  3. 2x2 limb products (one broadcast-AP int32 tensor_tensor) + middle sum.
  4. Base-4096 carry propagation in int32: L = t & 4095, carry = t >> 12.
  5. Bit extraction: one broadcast-AP tensor_tensor bitwise_and against a
     (1<<s) mask table, then ScalarE Sign (int32 -> f32 {0,1}) writes the
     output bits in DRAM row-major layout directly.

Engine balance per chunk: DVE does scans + conv + carry + extract-AND;
ScalarE does gathers + Sign; constants fill DVE's initial idle window; sync DMA moves
chunks (triple-buffered). All arithmetic is exact; rel err vs reference = 0.
"""

import numpy as np

import concourse.bass as bass
import concourse.bacc as bacc
import concourse.mybir as mybir
from concourse.bass_utils import run_bass_kernel_spmd
from concourse.tile import TileContext

F32 = mybir.dt.float32
I32 = mybir.dt.int32
OP = mybir.AluOpType
AF = mybir.ActivationFunctionType

P = 128            # SBUF partitions
N_CORES = 8
N_ROWS = 131072    # total batch
N_SHARD = N_ROWS // N_CORES  # 16384 rows per core
NCHUNK = 41        # selects CHUNK_SCHEDULES[41] = [16, 64, 32, 16]


CHUNK_SCHEDULES = {
    3: [16, 48, 64],
    4: [8, 24, 32, 32, 32],
    10: [32, 80, 16],
    20: [16, 48, 56, 8],
    21: [16, 40, 48, 16, 8],
    22: [8, 40, 56, 16, 8],
    11: [16, 96, 16],
    12: [48, 64, 16],
    13: [64, 48, 16],
    14: [24, 48, 48, 8],
    15: [32, 64, 24, 8],
    16: [48, 48, 24, 8],
    17: [96, 16, 16],
    18: [64, 32, 16, 16],
    30: [32, 48, 32, 16],
    31: [24, 48, 40, 16],
    32: [24, 40, 40, 16, 8],
    33: [8, 48, 48, 16, 8],
    34: [24, 48, 32, 16, 8],
    40: [16, 48, 48, 16],
    41: [16, 64, 32, 16],
    42: [24, 64, 24, 16],
    43: [32, 64, 16, 16],
    44: [16, 32, 48, 24, 8],
    45: [32, 48, 24, 16, 8],
    46: [24, 48, 24, 16, 16],
    47: [16, 48, 32, 24, 8],
    48: [16, 32, 64, 16],
    49: [16, 32, 48, 32],
}
GATHER_ON_ACT = True
SIGN_ON_ACT = True
TAIL_ROWS = 16
TAIL_LAG = 1       # tail pieces deferred behind this many newer pieces
LAST_TAIL_ROWS = 16  # tail-piece rows for the final chunk
MID_TAIL_ROWS = 16   # tail-piece rows for early/middle chunks
OUT_DMA_ON_ACT = False
OT_BUFS = 6
WORK_BUFS = 2
IO_BUFS = 3
VT_BUFS = 3


def _build_nc(n_rows: int, nchunk: int, repeats: int = 1) -> bass.Bass:
    R = n_rows // P          # rows per partition
    if nchunk in CHUNK_SCHEDULES and R == 128:
        chunks = CHUNK_SCHEDULES[nchunk]
    else:
        chunks = [R // nchunk] * nchunk
    assert sum(chunks) == R
    Rmax = max(chunks)
    FB = Rmax * 24           # input free elems per chunk (max)
    FO = Rmax * 48           # output free elems per chunk (max)

    nc = bacc.Bacc()
    A = nc.declare_dram_parameter("A", [n_rows, 24], F32, isOutput=False)
    B = nc.declare_dram_parameter("B", [n_rows, 24], F32, isOutput=False)
    O = nc.declare_dram_parameter("out", [n_rows, 48], F32, isOutput=True)

    # partition p <-> DRAM rows [p*R, (p+1)*R); contiguous per partition
    Av = A[:].rearrange("(p r) b -> p r b", p=P)
    Bv = B[:].rearrange("(p r) b -> p r b", p=P)
    Ov = O[:].rearrange("(p r) b -> p r b", p=P)

    with TileContext(nc) as tc:
        with (
            tc.tile_pool(name="const", bufs=1) as cpool,
            tc.tile_pool(name="io", bufs=IO_BUFS) as iopool,
            tc.tile_pool(name="work", bufs=WORK_BUFS) as wpool,
            tc.tile_pool(name="vt", bufs=VT_BUFS) as vtpool,
            tc.tile_pool(name="ot", bufs=OT_BUFS) as otpool,
        ):
            # scan multiplier pattern: 0 at 12-bit group starts, 0.5
            # elsewhere. Built on the VECTOR engine: DVE is idle during the
            # first input DMA anyway, this avoids a Pool->DVE cross-engine
            # dependency before the first scan, and sidesteps any GPSIMD
            # first-call IRAM-load cost inside the measured exec.
            p02 = cpool.tile([P, FB], F32, tag="p02")
            nc.vector.memset(p02[:], 0.5)
            nc.vector.memset(
                p02[:].rearrange("p (g e) -> p g e", e=12)[:, :, 0:1], 0.0
            )
            # mask tile [P, 48] int32: mask[k*12+s] = 1 << s
            mask = cpool.tile([P, 48], I32, tag="mask")
            mask3 = mask[:].rearrange("p (k s) -> p k s", s=12)
            for s in range(12):
                nc.vector.memset(mask3[:, :, s : s + 1], 1 << s)

            pend = []
            for rep_ in range(repeats):
              r0 = 0
              for ci_, Rc in enumerate(chunks):
                carry_over = (
                    0 if (ci_ == len(chunks) - 1 and rep_ == repeats - 1)
                    else TAIL_LAG
                )
                c0 = r0
                r0 += Rc
                a_t = iopool.tile([P, Rc * 24], F32, tag="a")
                b_t = iopool.tile([P, Rc * 24], F32, tag="b")
                nc.sync.dma_start(out=a_t[:], in_=Av[:, c0 : c0 + Rc, :])
                nc.sync.dma_start(out=b_t[:], in_=Bv[:, c0 : c0 + Rc, :])

                # 1. Horner scans -> scaled limbs (L * 2^-11) at positions 12l+11
                sa = wpool.tile([P, Rc * 24], F32, tag="sa")
                sb = wpool.tile([P, Rc * 24], F32, tag="sb")
                nc.vector.tensor_tensor_scan(
                    sa[:], p02[:, : Rc * 24], a_t[:], 0.0, OP.mult, OP.add
                )
                nc.vector.tensor_tensor_scan(
                    sb[:], p02[:, : Rc * 24], b_t[:], 0.0, OP.mult, OP.add
                )

                # 2. limb gathers on ScalarE: int limbs, limb-major [l][r]
                lai = wpool.tile([P, 2 * Rc], I32, tag="lai")
                lbi = wpool.tile([P, 2 * Rc], I32, tag="lbi")
                for src_, dst in ((sa, lai), (sb, lbi)):
                    # one op per input: iterate (r, l); in strides (24, 12)
                    # offset 11, out strides (1, Rc) -> limb-major [l][r]
                    sv = src_[:].rearrange("p (r l e) -> p r l e", l=2, e=12)[
                        :, :, :, 11
                    ]
                    dv = dst[:].rearrange("p (l r) -> p r l", l=2)
                    if GATHER_ON_ACT:
                        nc.scalar.activation(dv, sv, AF.Copy, scale=2048.0)
                    else:
                        nc.vector.tensor_scalar(dv, sv, 2048.0, None, OP.mult)

                # 3+4. conv products and carry propagation (int32).
                # Limb-source tile lt = [t0 | u1 | u2 | L3]: the extraction
                # masks only read bits 0..11, so raw column sums go in
                # unmasked (high bits are never looked at). DVE int32 math
                # runs in fp32 internally -> keep every result < 2^24:
                # products <= 4095^2, column sums < 2^14. All exact.
                lt = wpool.tile([P, 4 * Rc], I32, tag="lt")
                pt = wpool.tile([P, 3 * Rc], I32, tag="pt")  # a0b1, a1b0, a1b1
                nc.vector.tensor_tensor(
                    lt[:, 0:Rc], lai[:, 0:Rc], lbi[:, 0:Rc], OP.mult
                )
                nc.vector.tensor_tensor(
                    pt[:, 0:Rc], lai[:, 0:Rc], lbi[:, Rc : 2 * Rc], OP.mult
                )
                nc.vector.tensor_tensor(
                    pt[:].rearrange("p (j r) -> p j r", j=3)[:, 1:3, :],
                    lai[:, Rc : 2 * Rc].unsqueeze(1).broadcast_to([P, 2, Rc]),
                    lbi[:].rearrange("p (j r) -> p j r", j=2),
                    OP.mult,
                )
                bd = wpool.tile([P, 3 * Rc], I32, tag="bd")  # lo12 of pt
                bs = wpool.tile([P, 3 * Rc], I32, tag="bs")  # hi12 of pt
                nc.vector.tensor_scalar(bd[:], pt[:], 4095, None, OP.bitwise_and)
                nc.vector.tensor_scalar(bs[:], pt[:], 12, None, OP.arith_shift_right)
                k0 = wpool.tile([P, Rc], I32, tag="k0")
                nc.vector.tensor_scalar(
                    k0[:], lt[:, 0:Rc], 12, None, OP.arith_shift_right
                )
                u1 = wpool.tile([P, Rc], I32, tag="u1")
                nc.vector.tensor_tensor(u1[:], bd[:, 0:Rc], bd[:, Rc : 2 * Rc], OP.add)
                nc.vector.tensor_tensor(lt[:, Rc : 2 * Rc], u1[:], k0[:], OP.add)
                k1a = wpool.tile([P, Rc], I32, tag="k1a")
                nc.vector.tensor_scalar(
                    k1a[:], lt[:, Rc : 2 * Rc], 12, None, OP.arith_shift_right
                )
                k1b = wpool.tile([P, Rc], I32, tag="k1b")
                nc.vector.tensor_tensor(k1b[:], k1a[:], bs[:, 0:Rc], OP.add)
                k1 = wpool.tile([P, Rc], I32, tag="k1")
                nc.vector.tensor_tensor(k1[:], k1b[:], bs[:, Rc : 2 * Rc], OP.add)
                nc.vector.tensor_tensor(
                    lt[:, 2 * Rc : 3 * Rc], bd[:, 2 * Rc :], k1[:], OP.add
                )
                k2 = wpool.tile([P, Rc], I32, tag="k2")
                nc.vector.tensor_scalar(
                    k2[:], lt[:, 2 * Rc : 3 * Rc], 12, None, OP.arith_shift_right
                )
                nc.vector.tensor_tensor(
                    lt[:, 3 * Rc :], k2[:], bs[:, 2 * Rc :], OP.add
                )

                # 5. bit extraction in tail pieces (<=TAIL_ROWS rows each):
                #    (L_k & (1<<s)) now; Sign + DMA-out deferred one chunk so
                #    ScalarE always runs the next chunk's gathers (on DVE's
                #    critical path) before the previous chunk's Sign tail.
                last_chunk = ci_ == len(chunks) - 1 and rep_ == repeats - 1
                near_end = last_chunk or ci_ == len(chunks) - 2
                tr = (
                    LAST_TAIL_ROWS if last_chunk
                    else (TAIL_ROWS if near_end else MID_TAIL_ROWS)
                )
                npc = max(1, Rc // tr)
                assert Rc % npc == 0, (Rc, npc)
                Rh = Rc // npc
                ltv = lt[:].rearrange("p (k r) -> p k r", k=4)
                for h in range(npc):
                    vt = vtpool.tile([P, Rh * 48], I32, tag=f"vt{h % 4}")
                    lt4 = (
                        ltv[:, :, h * Rh : (h + 1) * Rh]
                        .transpose([0, 2, 1])
                        .unsqueeze(3)
                        .broadcast_to([P, Rh, 4, 12])
                    )
                    mask4 = mask3.unsqueeze(1).broadcast_to([P, Rh, 4, 12])
                    nc.vector.tensor_tensor(
                        vt[:].rearrange("p (r k s) -> p r k s", k=4, s=12),
                        lt4,
                        mask4,
                        OP.bitwise_and,
                    )
                    pend.append((vt, c0 + h * Rh, Rh))
                while len(pend) > carry_over:
                    vt, row0, Rh_ = pend.pop(0)
                    o_t = otpool.tile([P, Rh_ * 48], F32, tag="o")
                    # very last piece: compare on DVE to skip the DVE->ACT hop
                    last_piece = carry_over == 0 and not pend
                    if SIGN_ON_ACT and not last_piece:
                        nc.scalar.activation(o_t[:], vt[:], AF.Sign)
                    else:
                        nc.vector.tensor_scalar(o_t[:], vt[:], 0, None, OP.is_gt)
                    if OUT_DMA_ON_ACT:
                        nc.scalar.dma_start(
                            out=Ov[:, row0 : row0 + Rh_, :], in_=o_t[:]
                        )
                    else:
                        nc.sync.dma_start(
                            out=Ov[:, row0 : row0 + Rh_, :], in_=o_t[:]
                        )

    nc.finalize()
    return nc


_CACHE = {}


def _get_nc():
    key = (N_SHARD, NCHUNK)
    if key not in _CACHE:
        _CACHE[key] = _build_nc(N_SHARD, NCHUNK)
    return _CACHE[key]


def kernel(A: np.ndarray, B: np.ndarray) -> np.ndarray:
    A = np.ascontiguousarray(A, dtype=np.float32)
    B = np.ascontiguousarray(B, dtype=np.float32)
    nc = _get_nc()
    in_maps = [
        {
            "A": A[c * N_SHARD : (c + 1) * N_SHARD],
            "B": B[c * N_SHARD : (c + 1) * N_SHARD],
        }
        for c in range(N_CORES)
    ]
    res = run_bass_kernel_spmd(nc, in_maps, core_ids=list(range(N_CORES)))
    return np.concatenate([res.results[i]["out"] for i in range(N_CORES)], axis=0)



# revision 2
# speedup vs baseline: 1.0327x; 1.0327x over previous
"""Trainium2 Bass kernel: 24x24-bit array multiplier (bit-vector in/out).

Inputs  A, B: [131072, 24] f32 {0,1} bits, LSB-first.
Output: [131072, 48] f32 {0,1} product bits, LSB-first.

Pure data-parallel across 8 NeuronCores (16384 rows each). Per core,
per chunk of rows, with A and B halves packed in ONE tile so every
stage is a single instruction over both operands:

  1. COMBINE_LEVELS scalar_tensor_tensor passes (mult/add, DVE 2x_2p
     rate) fold bit pairs into base-4 then base-16 digits
     (digit = base*odd + even).
  2. One short Horner scan (tensor_tensor_scan, multiplier 1/16, reset
     0 at group starts) turns digits into scaled 12-bit limbs.
  3. One limb gather on ScalarE (activation Copy, scale 16^2,
     f32->int32) -> [a0|a1|b0|b1] limb-major.
  4. Two scalar_tensor_tensor (bypass, mult) products ->
     pt = [p00|p10|p01|p11], then an 8-instruction exact base-4096
     carry chain that overwrites pt in place to [t0|t1|t2|t3]:
       t1 = p01 + (p10 & fff) + (p00 >> 12)   (fits < 2^24 exactly)
       t2 = p11 + (p10 >> 12) + (t1 >> 12)    (fits < 2^24 exactly)
       t3 = t2 >> 12
     Digits keep junk bits >= 12; extraction never reads them. All
     values < 2^24 so DVE int32 (fp32-internal) math is exact.
  5. Bit extraction: 12 single-op tensor_scalar ANDs (digit & (1<<s))
     at the DVE 2x_2p rate write int32 {0, 2^s} in DRAM row-major
     order; ScalarE Sign converts to f32 {0,1} per piece; per-piece
     output DMA. Input DMAs are all prefetched on SP so no output wait
     blocks them.

Constants (scan multiplier pattern) are built on the otherwise-idle
Pool/GPSIMD engine during the first input DMA.

All arithmetic is exact; rel err vs reference = 0.
"""

import numpy as np

import concourse.bass as bass
import concourse.bacc as bacc
import concourse.mybir as mybir
from concourse.bass_utils import run_bass_kernel_spmd
from concourse.tile import TileContext

F32 = mybir.dt.float32
I32 = mybir.dt.int32
OP = mybir.AluOpType
AF = mybir.ActivationFunctionType

P = 128            # SBUF partitions
N_CORES = 8
N_ROWS = 131072    # total batch
N_SHARD = N_ROWS // N_CORES  # 16384 rows per core

CHUNKS = [24, 40, 40, 24]  # rows/partition per chunk; sum must be 128
COMBINE_LEVELS = 0      # 0: scan raw bits; 1: base-4 digits; 2: base-16 digits
P02_ON_POOL = True      # build scan-multiplier constant on GPSIMD
GATHER_ON_ACT = False   # limb gather on ScalarE (else DVE tensor_scalar)
PREFETCH_ALL = True     # issue every input DMA before any output DMA
B_FIRST = False         # DMA the B half before the A half
SIGN_PIECES = 2         # Sign+out-DMA pieces per chunk
LAST_SIGN_ON_DVE = True  # last piece: is_gt on DVE instead of ACT Sign
XT_BUFS = 3             # buffers for the extraction staging pool
EXTR_PRIO = 0           # high_priority offset for extraction+sign+dma (0 = off)
SPLIT_FIRST = True      # chunk 0: scan A and B separately (starts earlier)
STAGGER_US = 4.5        # tile_wait_until stagger per chunk for scans (µs)


def _build_nc(n_rows: int, chunks=None, repeats: int = 1) -> bass.Bass:
    R = n_rows // P          # rows per partition
    chunks = chunks or CHUNKS
    assert sum(chunks) == R, (chunks, R)
    Rmax = max(chunks)

    nc = bacc.Bacc()
    A = nc.declare_dram_parameter("A", [n_rows, 24], F32, isOutput=False)
    B = nc.declare_dram_parameter("B", [n_rows, 24], F32, isOutput=False)
    O = nc.declare_dram_parameter("out", [n_rows, 48], F32, isOutput=True)

    # partition p <-> DRAM rows [p*R, (p+1)*R); contiguous per partition
    Av = A[:].rearrange("(p r) b -> p r b", p=P)
    Bv = B[:].rearrange("(p r) b -> p r b", p=P)
    Ov = O[:].rearrange("(p r) b -> p r b", p=P)

    with TileContext(nc) as tc:
        with (
            tc.tile_pool(name="const", bufs=1) as cpool,
            tc.tile_pool(name="io", bufs=max(3, len(chunks))) as iopool,
            tc.tile_pool(name="work", bufs=3) as wpool,
            tc.tile_pool(name="xt", bufs=XT_BUFS) as xtpool,
            tc.tile_pool(
                name="ot",
                bufs=min(
                    len(chunks) * SIGN_PIECES + 1,
                    max(3, 36864 // ((Rmax // SIGN_PIECES) * 48 * 4)),
                ),
            ) as otpool,
        ):
            LV = COMBINE_LEVELS
            GE = 12 >> LV                       # scan group length
            SW = 24 >> LV                       # scan elems per row
            MULT = 1.0 / (1 << (1 << LV))       # 0.5 / 0.25 / 0.0625
            GSCALE = float((1 << (1 << LV)) ** (GE - 1))
            eng0 = nc.gpsimd if P02_ON_POOL else nc.vector
            p02 = cpool.tile([P, 2 * Rmax * SW], F32, tag="p02")
            eng0.memset(p02[:], MULT)
            eng0.memset(
                p02[:].rearrange("p (g e) -> p g e", e=GE)[:, :, 0:1], 0.0
            )

            # up-front input DMAs (SP sequencer never blocks on compute)
            ab_tiles = []
            c0 = 0
            for ci, Rc in enumerate(chunks):
                ab_t = iopool.tile([P, 2 * Rc * 24], F32, tag=f"ab{ci}")
                halves = [
                    (ab_t[:, 0 : Rc * 24], Av),
                    (ab_t[:, Rc * 24 : 2 * Rc * 24], Bv),
                ]
                if B_FIRST:
                    halves.reverse()
                for dst, srcv in halves:
                    nc.sync.dma_start(out=dst, in_=srcv[:, c0 : c0 + Rc, :])
                ab_tiles.append(ab_t)
                c0 += Rc

            c0 = 0
            for ci, Rc in enumerate(chunks):
                last_chunk = ci == len(chunks) - 1
                ab_t = ab_tiles[ci]

                # 1+2. combines then one Horner scan over both halves
                x_in = ab_t
                for lv in range(LV):
                    w = 2 * Rc * (24 >> (lv + 1))
                    base = float(1 << (1 << lv))
                    cc = wpool.tile([P, w], F32, tag=f"cc{lv}")
                    v2 = x_in[:].rearrange("p (j two) -> p j two", two=2)
                    nc.vector.scalar_tensor_tensor(
                        cc[:].unsqueeze(2), v2[:, :, 1:2], base,
                        v2[:, :, 0:1], OP.mult, OP.add,
                    )
                    x_in = cc
                ss = wpool.tile([P, 2 * Rc * SW], F32, tag="ss")
                if STAGGER_US:
                    tc.tile_set_cur_wait(ci * STAGGER_US / 1000.0)
                if SPLIT_FIRST and ci == 0:
                    # separate scans per operand half: the first starts as
                    # soon as its own DMA lands
                    hw_ = Rc * SW
                    for off in (0, hw_):
                        nc.vector.tensor_tensor_scan(
                            ss[:, off : off + hw_], p02[:, :hw_],
                            x_in[:, off : off + hw_], 0.0, OP.mult, OP.add,
                        )
                else:
                    nc.vector.tensor_tensor_scan(
                        ss[:], p02[:, : 2 * Rc * SW], x_in[:], 0.0,
                        OP.mult, OP.add,
                    )

                # 3. limb gather -> li = [a0|a1|b0|b1], int32 limb-major
                li = wpool.tile([P, 4 * Rc], I32, tag="li")
                sv = ss[:].rearrange(
                    "p (h r l e) -> p h l r e", h=2, l=2, e=GE
                )[:, :, :, :, GE - 1]
                dv = li[:].rearrange("p (h l r) -> p h l r", h=2, l=2)
                if GATHER_ON_ACT and not last_chunk:
                    nc.scalar.activation(dv, sv, AF.Copy, scale=GSCALE)
                else:
                    nc.vector.tensor_scalar(dv, sv, GSCALE, None, OP.mult)

                # 4. products pt = [p00|p10|p01|p11] then in-place carry
                # chain -> [t0|t1|t2|t3]
                pt = wpool.tile([P, 4 * Rc], I32, tag="pt")
                pv = pt[:].rearrange("p (k r) -> p k r", k=4)
                aa = li[:, 0 : 2 * Rc].rearrange("p (l r) -> p l r", l=2)
                b0 = li[:, 2 * Rc : 3 * Rc].unsqueeze(1)
                b1 = li[:, 3 * Rc : 4 * Rc].unsqueeze(1)
                nc.vector.scalar_tensor_tensor(
                    pv[:, 0:2, :], aa, 0, b0.broadcast_to([P, 2, Rc]),
                    OP.bypass, OP.mult,
                )
                nc.vector.scalar_tensor_tensor(
                    pv[:, 2:4, :], aa, 0, b1.broadcast_to([P, 2, Rc]),
                    OP.bypass, OP.mult,
                )
                sc = wpool.tile([P, 4 * Rc], I32, tag="sc")
                kh = sc[:, 0 : 2 * Rc].rearrange("p (k r) -> p k r", k=2)
                k0 = kh[:, 0:1, :]
                h10 = kh[:, 1:2, :]
                l10 = sc[:, 2 * Rc : 3 * Rc].unsqueeze(1)
                m1 = sc[:, 3 * Rc : 4 * Rc].unsqueeze(1)
                # [k0, h10] = [p00, p10] >> 12 ; l10 = p10 & fff
                nc.vector.tensor_scalar(
                    kh, pv[:, 0:2, :], 12, None, OP.arith_shift_right
                )
                nc.vector.tensor_scalar(
                    l10, pv[:, 1:2, :], 4095, None, OP.bitwise_and
                )
                # t1 = p01 + (l10 + k0)   (slot1; p10 dead)
                nc.vector.scalar_tensor_tensor(
                    m1, l10, 0, k0, OP.bypass, OP.add
                )
                nc.vector.scalar_tensor_tensor(
                    pv[:, 1:2, :], pv[:, 2:3, :], 0, m1, OP.bypass, OP.add
                )
                # t2 = p11 + (h10 + (t1 >> 12))   (slot2; p01 dead)
                nc.vector.tensor_scalar(
                    l10, pv[:, 1:2, :], 12, None, OP.arith_shift_right
                )
                nc.vector.scalar_tensor_tensor(
                    m1, h10, 0, l10, OP.bypass, OP.add
                )
                nc.vector.scalar_tensor_tensor(
                    pv[:, 2:3, :], pv[:, 3:4, :], 0, m1, OP.bypass, OP.add
                )
                # t3 = t2 >> 12   (slot3; p11 dead)
                nc.vector.tensor_scalar(
                    pv[:, 3:4, :], pv[:, 2:3, :], 12, None,
                    OP.arith_shift_right,
                )

                # 5. extraction: 12 single-op ANDs -> int32 {0, 2^s} in
                # DRAM row-major order; Sign per piece -> f32; per-piece
                # output DMA.
                import contextlib
                prio_ctx = (
                    tc.high_priority(offset=EXTR_PRIO)
                    if EXTR_PRIO else contextlib.nullcontext()
                )
                with prio_ctx:
                    xt = xtpool.tile([P, Rc * 48], I32, tag="x")
                    d4 = pt[:].rearrange("p (k r) -> p r k", k=4)
                    xv4 = xt[:].rearrange("p (r k s) -> p r k s", k=4, s=12)
                    for s in range(12):
                        nc.vector.tensor_scalar(
                            xv4[:, :, :, s], d4, 1 << s, None, OP.bitwise_and
                        )
                    npc = SIGN_PIECES if Rc % SIGN_PIECES == 0 else 1
                    Rh = Rc // npc
                    for h in range(npc):
                        o_t = otpool.tile([P, Rh * 48], F32, tag="o")
                        xs = xt[:, h * Rh * 48 : (h + 1) * Rh * 48]
                        last_piece = last_chunk and h == npc - 1
                        if LAST_SIGN_ON_DVE and last_piece:
                            nc.vector.tensor_scalar(
                                o_t[:], xs, 0, None, OP.is_gt
                            )
                        else:
                            nc.scalar.activation(o_t[:], xs, AF.Sign)
                        nc.sync.dma_start(
                            out=Ov[:, c0 + h * Rh : c0 + (h + 1) * Rh, :],
                            in_=o_t[:],
                        )
                c0 += Rc

    nc.finalize()
    return nc


_CACHE = {}


def _get_nc():
    key = (N_SHARD, tuple(CHUNKS))
    if key not in _CACHE:
        _CACHE[key] = _build_nc(N_SHARD)
    return _CACHE[key]


def kernel(A: np.ndarray, B: np.ndarray) -> np.ndarray:
    A = np.ascontiguousarray(A, dtype=np.float32)
    B = np.ascontiguousarray(B, dtype=np.float32)
    nc = _get_nc()
    in_maps = [
        {
            "A": A[c * N_SHARD : (c + 1) * N_SHARD],
            "B": B[c * N_SHARD : (c + 1) * N_SHARD],
        }
        for c in range(N_CORES)
    ]
    res = run_bass_kernel_spmd(nc, in_maps, core_ids=list(range(N_CORES)))
    return np.concatenate([res.results[i]["out"] for i in range(N_CORES)], axis=0)
